# revision 7
# baseline (speedup 1.0000x reference)
"""CvT attention block, whole-body For_i(reps) hardware-loop variant.

Every rep executes as one iteration of a single For_i loop over a fixed-AP
body: all tiles are allocated once and each iteration recomputes the full
kernel in place. Iterations of a resident loop body dispatch far cheaper
than unrolled instructions on this execution path.

Dtype split tuned to the dispatch cost model: the two dominant matmul
groups (scores, attn@V) use f32r operands, whose matmuls self-load weights
(one instruction each); the small projection groups stay bf16 (legalized
into Ldweights+Matmult pairs) to keep the everything-resident SBUF layout
under budget. Scores and AV accumulators live in disjoint, bank-aligned
regions of one PSUM slab so AV does not WAR-serialize against the score
tile readers.
"""

import contextlib
import numpy as np
import ml_dtypes
from concourse import mybir
import concourse.bacc as bacc
import concourse.tile as tile
from concourse.bass_utils import run_bass_kernel_spmd

F32 = mybir.dt.float32
F32R = mybir.dt.float32r
BF16 = mybir.dt.bfloat16
AFT = mybir.ActivationFunctionType
ALU = mybir.AluOpType

C = 384
T = 3136
TKV = 784
NH = 6
SCALE = C ** (-0.5)
EPS = 1e-5
GP = 3364
LH = [(0, 1536), (1536, 1600)]

_CACHE = {}


def _windows(total, w):
    return [(o, min(w, total - o)) for o in range(0, total, w)]


def _conv(nc, x3, ys, yout, wb, cv, stride):
    w = lambda t: wb[:, 9 * cv + t:9 * cv + t + 1]
    bias = wb[:, 27 + cv:28 + cv]
    sl = lambda di, dj: x3[:, 1 + di:57 + di:stride, 1 + dj:57 + dj:stride]
    nc.vector.tensor_scalar(ys[:], sl(-1, -1), w(0), bias,
                            op0=ALU.mult, op1=ALU.add)
    for t in range(1, 8):
        di, dj = t // 3 - 1, t % 3 - 1
        nc.vector.scalar_tensor_tensor(ys[:], sl(di, dj), w(t), ys[:],
                                       op0=ALU.mult, op1=ALU.add)
    nc.vector.scalar_tensor_tensor(yout[:], sl(1, 1), w(8), ys[:],
                                   op0=ALU.mult, op1=ALU.add)


def _body(nc, tc, d, wq, wk, wvp, wpj, wb, bpj3, xt, yq, yk, yv, QT, KT, Vh,
          OT2, attnT, Dh, D1, outT, slab):
    # ---- Phase A ----
    for ch in range(3):
        nc.sync.dma_start(xt[:], d["xp"][:, ch, :])
        x3 = xt[:].rearrange("p (r c) -> p r c", c=58)
        ysl = slab()
        _conv(nc, x3, ysl[:, 0:T], yq[ch], wb[ch], 0, 1)
        ysl = slab()
        _conv(nc, x3, ysl[:, 0:TKV], yk[ch], wb[ch], 1, 2)
        ysl = slab()
        _conv(nc, x3, ysl[:, 0:TKV], yv[ch], wb[ch], 2, 2)

    # ---- Phase B ----
    for co in range(3):
        p = slab()
        for lo, ls in _windows(T, 512):
            for ch in range(3):
                nc.tensor.matmul(p[:, lo:lo + ls],
                                 wq[ch][:, co * 128:(co + 1) * 128],
                                 yq[ch][:, lo:lo + ls],
                                 start=(ch == 0), stop=(ch == 2))
        nc.scalar.activation(QT[co][:], p[:, 0:T], AFT.Copy)
        p = slab()
        for lo, ls in _windows(TKV, 512):
            for ch in range(3):
                nc.tensor.matmul(p[:, lo:lo + ls],
                                 wk[ch][:, co * 128:(co + 1) * 128],
                                 yk[ch][:, lo:lo + ls],
                                 start=(ch == 0), stop=(ch == 2))
        nc.scalar.activation(KT[co][:], p[:, 0:TKV], AFT.Copy)
    p = slab()
    for tt in range(7):
        for ch in range(3):
            nc.tensor.matmul(p[0:112, tt * 512:tt * 512 + 390],
                             yv[ch][:, tt * 112:(tt + 1) * 112],
                             wvp[ch][:], start=(ch == 0), stop=(ch == 2))
    pv = p[0:112, 0:3584].rearrange("p (w c) -> p w c", c=512)[:, :, 0:390]
    nc.scalar.activation(Vh[:].rearrange("p (w c) -> p w c", c=390), pv,
                         AFT.Copy)
    nc.vector.memset(
        Vh[:].rearrange("p (g c) -> p g c", c=65)[:, :, 64:65]
        .bitcast(mybir.dt.uint32), 0x3F800000)

    # ---- Phase C ----
    for h in range(NH):
        c2, po = h // 2, 64 * (h % 2)
        for lo, lw in LH:
            # one slab rotation per (head, l-half); scores and AV live in
            # disjoint regions so AV does not WAR-alias the score tile
            reg = slab()
            pS = reg[:, 0:1600]
            pO = reg[:, 2048:3648]
            for tt in range(7):
                for o, w_ in _windows(lw, 512):
                    nc.tensor.matmul(
                        pS[0:112, o:o + w_],
                        KT[c2][po:po + 64, tt * 112:(tt + 1) * 112],
                        QT[c2][po:po + 64, lo + o:lo + o + w_],
                        start=True, stop=True)
                nc.scalar.activation(attnT[:, tt, 0:lw], pS[0:112, 0:lw],
                                     AFT.Exp, scale=float(SCALE))
            for tt in range(7):
                for o, w_ in _windows(lw, 512):
                    nc.tensor.matmul(
                        pO[0:65, o:o + w_],
                        Vh[:, tt * 390 + h * 65:tt * 390 + (h + 1) * 65],
                        attnT[:, tt, o:o + w_],
                        start=(tt == 0), stop=(tt == 6))
            nc.scalar.activation(OT2[po:po + 64, c2, lo:lo + lw],
                                 pO[0:64, 0:lw], AFT.Copy)
            with nc.allow_low_precision(reason="recip"):
                nc.vector.reciprocal(D1[:, 0:lw], pO[64:65, 0:lw])
            nc.gpsimd.partition_broadcast(Dh[:, 0:lw], D1[:, 0:lw])
            nc.vector.tensor_mul(OT2[po:po + 64, c2, lo:lo + lw],
                                 OT2[po:po + 64, c2, lo:lo + lw],
                                 Dh[po:po + 64, 0:lw])

    # ---- Phase D ----
    for co in range(3):
        pP = slab()
        for lo, ls in _windows(T, 512):
            for c2 in range(3):
                nc.tensor.matmul(pP[:, lo:lo + ls],
                                 wpj[c2][:, co * 128:(co + 1) * 128],
                                 OT2[:, c2, lo:lo + ls],
                                 start=(c2 == 0), stop=(c2 == 2))
        nc.scalar.activation(outT[:, co, :], pP[:, 0:T], AFT.Identity,
                             bias=bpj3[:, co:co + 1])
    nc.sync.dma_start(d["out"], outT[:])


def _emit(nc, tc, ctx, d, reps):
    pers = ctx.enter_context(tc.tile_pool(name="pers", bufs=1))
    wq = [pers.tile([128, C], BF16, tag=f"wq{i}", name=f"wq{i}") for i in range(3)]
    wk = [pers.tile([128, C], BF16, tag=f"wk{i}", name=f"wk{i}") for i in range(3)]
    wvp = [pers.tile([128, NH * 65], BF16, tag=f"wvp{i}", name=f"wvp{i}")
           for i in range(3)]
    wpj = [pers.tile([128, C], BF16, tag=f"wpj{i}", name=f"wpj{i}")
           for i in range(3)]
    wb = [pers.tile([128, 30], F32, tag=f"wb{i}", name=f"wb{i}")
          for i in range(3)]
    bpj3 = pers.tile([128, 3], F32, tag="bpj3", name="bpj3")

    for i in range(3):
        nc.sync.dma_start(wq[i][:], d["wq"][i * 128:(i + 1) * 128, :])
        nc.sync.dma_start(wk[i][:], d["wk"][i * 128:(i + 1) * 128, :])
        nc.sync.dma_start(wvp[i][:], d["wvp"][i * 128:(i + 1) * 128, :])
        nc.sync.dma_start(wpj[i][:], d["wpj"][i * 128:(i + 1) * 128, :])
        nc.sync.dma_start(wb[i][:], d["wb"][i])
    nc.sync.dma_start(bpj3[:], d["bpj3"])

    work = ctx.enter_context(tc.tile_pool(name="work", bufs=1))
    psp = ctx.enter_context(tc.tile_pool(name="ps", bufs=1, space="PSUM"))
    slab = lambda: psp.tile([128, 4096], F32, tag="slab", name="slab")

    xt = work.tile([128, GP], BF16, tag="xt", name="xt")
    yq = [work.tile([128, T], BF16, tag=f"yq{i}", name=f"yq{i}")
          for i in range(3)]
    yk = [work.tile([128, TKV], BF16, tag=f"yk{i}", name=f"yk{i}")
          for i in range(3)]
    yv = [work.tile([128, TKV], BF16, tag=f"yv{i}", name=f"yv{i}")
          for i in range(3)]
    QT = [work.tile([128, T], F32R, tag=f"QT{i}", name=f"QT{i}")
          for i in range(3)]
    KT = [work.tile([128, TKV], F32R, tag=f"KT{i}", name=f"KT{i}")
          for i in range(3)]
    Vh = work.tile([112, 7 * 390], F32R, tag="Vh", name="Vh")
    OT2 = work.tile([128, 3, T], BF16, tag="OT2", name="OT2")
    attnT = work.tile([112, 7, 1600], F32R, tag="attnT", name="attnT")
    Dh = work.tile([128, 1600], BF16, tag="Dh", name="Dh")
    D1 = work.tile([1, 1600], BF16, tag="D1", name="D1")
    outT = work.tile([128, 3, T], BF16, tag="outT", name="outT")

    args = (nc, tc, d, wq, wk, wvp, wpj, wb, bpj3, xt, yq, yk, yv, QT, KT,
            Vh, OT2, attnT, Dh, D1, outT, slab)
    if reps == 1:
        _body(*args)
    else:
        with tc.For_i(0, reps):
            _body(*args)


def _build(reps=1):
    if reps in _CACHE:
        return _CACHE[reps]
    nc = bacc.Bacc("TRN2", target_bir_lowering=False, debug=False)
    d = {
        "xp": nc.dram_tensor("xp", [128, 3, GP], BF16, kind="ExternalInput").ap(),
        "wb": nc.dram_tensor("wb", [3, 128, 30], F32, kind="ExternalInput").ap(),
        "wq": nc.dram_tensor("wq", [C, C], BF16, kind="ExternalInput").ap(),
        "wk": nc.dram_tensor("wk", [C, C], BF16, kind="ExternalInput").ap(),
        "wvp": nc.dram_tensor("wvp", [C, NH * 65], BF16,
                              kind="ExternalInput").ap(),
        "wpj": nc.dram_tensor("wpj", [C, C], BF16, kind="ExternalInput").ap(),
        "bpj3": nc.dram_tensor("bpj3", [128, 3], F32, kind="ExternalInput").ap(),
        "out": nc.dram_tensor("out", [128, 3, T], BF16,
                              kind="ExternalOutput").ap(),
    }
    with tile.TileContext(nc) as tc:
        with contextlib.ExitStack() as ctx:
            _emit(nc, tc, ctx, d, reps)
    nc.compile()
    _CACHE[reps] = nc
    return nc


def _host_prep(x, conv_q, conv_k, conv_v, bn_q, bn_k, bn_v, Wq, Wk, Wv,
               Wproj, bproj):
    B = x.shape[0]
    x = np.asarray(x, np.float32)
    xp = np.zeros((B, C, 58, 58), np.float32)
    xp[:, :, 1:57, 1:57] = x.transpose(0, 2, 1).reshape(B, C, 56, 56)
    xp = np.ascontiguousarray(
        xp.reshape(B, 3, 128, GP).transpose(0, 2, 1, 3)).astype(
        ml_dtypes.bfloat16)

    wb = np.zeros((3, 128, 30), np.float32)
    for cv, (w, bn) in enumerate(((conv_q, bn_q), (conv_k, bn_k),
                                  (conv_v, bn_v))):
        g, b, m, v = [np.asarray(bn[i], np.float64) for i in range(4)]
        a = g / np.sqrt(v + EPS)
        bias = (b - m * a).astype(np.float32)
        wh = (np.asarray(w, np.float64).reshape(C, 9) * a[:, None]).astype(
            np.float32)
        for ch in range(3):
            wb[ch, :, 9 * cv:9 * cv + 9] = wh[ch * 128:(ch + 1) * 128]
            wb[ch, :, 27 + cv] = bias[ch * 128:(ch + 1) * 128]

    bf = ml_dtypes.bfloat16
    wvp = np.zeros((C, NH * 65), np.float32)
    Wv = np.asarray(Wv, np.float32)
    for h in range(NH):
        wvp[:, h * 65:h * 65 + 64] = Wv[:, h * 64:(h + 1) * 64]

    bpj3 = np.zeros((128, 3), np.float32)
    bproj = np.asarray(bproj, np.float32)
    for co in range(3):
        bpj3[:, co] = bproj[co * 128:(co + 1) * 128]

    return {
        "xp": xp,
        "wb": wb,
        "wq": np.asarray(Wq, np.float32).astype(bf),
        "wk": np.asarray(Wk, np.float32).astype(bf),
        "wvp": wvp.astype(bf),
        "wpj": np.asarray(Wproj, np.float32).astype(bf),
        "bpj3": bpj3,
    }


def kernel(x, h, w, conv_q, conv_k, conv_v, bn_q, bn_k, bn_v, Wq, Wk, Wv,
           Wproj, bproj, _reps=1, _nc=None):
    B = x.shape[0]
    nc = _nc if _nc is not None else _build(_reps)
    hp = _host_prep(x, conv_q, conv_k, conv_v, bn_q, bn_k, bn_v, Wq, Wk, Wv,
                    Wproj, bproj)
    shared = {k: v for k, v in hp.items() if k != "xp"}
    in_maps = [dict(shared, xp=hp["xp"][b]) for b in range(B)]
    res = run_bass_kernel_spmd(nc, in_maps, core_ids=list(range(B)))
    out = np.stack([res.results[b]["out"] for b in range(B)], axis=0)
    return np.ascontiguousarray(out.transpose(0, 3, 2, 1)).reshape(
        B, T, C).astype(np.float32)
